# revision 1
# baseline (speedup 1.0000x reference)
"""ConvCharEmbedding Trainium2 kernel.

Reference computation (per word of C=16 chars):
    e = emb[ids]                # [C, E] gather
    y = conv1d(e.T, W, pad=2)   # [E, C], kernel K=5
    out = max_c(y + b)          # [E]

Device algorithm (per core, data-parallel over batch):
  * embedding table pre-cast to bf16 on host, padded to 264 rows
    (row 262 = zeros, used for conv padding slots), resident in DRAM.
  * char ids padded to 20 slots/word (2 zero-row slots each side);
    one batched SWDGE dma_gather (transpose mode) per 512-word chunk
    fetches all 10240 slot embeddings and writes them directly as
    e^T [E=128 partitions, word, slot] bf16 in SBUF - no PE transposes.
  * conv = 5 PSUM-accumulated bf16 matmuls; lhsT = W_k^T [i, o] (bf16),
    rhs = shifted windows of e^T, 512 output positions (32 words x 16
    chars) per PSUM group.
  * DVE max-reduce over the 16 char positions straight out of PSUM,
    ACT bias add, DMA out in [o, n] layout; host transposes.
"""

import numpy as np

import concourse.bass as bass
import concourse.tile as tile
from concourse import bacc, mybir
from concourse.bass_utils import run_bass_kernel_spmd

B, W, C = 128, 256, 16
E = 128
K = 5
PAD = 2
V = 262
VZERO = 262                   # zero row index in the padded table
VROWS = 264                   # table rows (262 real + 2 zero)
NCORES = 8
B_SH = B // NCORES            # 16 batches per core
N = B_SH * W                  # 4096 words per core
SLOTS = C + 2 * PAD           # 20 slots per word (padded)
CHUNK_N = 512                 # words per gather chunk
NCHUNK = N // CHUNK_N         # 8
IDX_CHUNK = CHUNK_N * SLOTS   # 10240 gather indices per chunk
IDXCOL = IDX_CHUNK // 16      # 640 idx columns per chunk (16-partition wrap)
GROUP_N = 32                  # words per PSUM group (32*16 = 512 positions)
NGROUP = CHUNK_N // GROUP_N   # 16 groups per chunk
NQUEUES = 4                   # SWDGE queues to round-robin gathers over
G_IDX = 640                   # idxs per dma_gather (SWDGE ring caps ~1024)
NSUB = IDX_CHUNK // G_IDX     # 16 sub-gathers per chunk

dt = mybir.dt


def build_program():
    nc = bacc.Bacc("TRN2", target_bir_lowering=False, debug=False,
                   num_devices=NCORES, dynamic_dma_scratch_size=16384,
                   num_swdge_queues=NQUEUES)

    ids_d = nc.dram_tensor("ids", [128, NCHUNK * IDXCOL], dt.int16,
                           kind="ExternalInput")
    emb_d = nc.dram_tensor("embtab", [VROWS, E], dt.bfloat16,
                           kind="ExternalInput")
    wt_d = nc.dram_tensor("wt", [E, K * E], dt.bfloat16, kind="ExternalInput")
    convb_d = nc.dram_tensor("convb", [E, 1], dt.float32, kind="ExternalInput")
    out_d = nc.dram_tensor("out", [E, N], dt.float32, kind="ExternalOutput")

    with tile.TileContext(nc) as tc:
        with (
            tc.tile_pool(name="const", bufs=1) as const_pool,
            tc.tile_pool(name="et", bufs=2) as et_pool,
            tc.tile_pool(name="yc", bufs=2) as y_pool,
            tc.tile_pool(name="ps", bufs=8, space="PSUM") as ps_pool,
        ):
            # ---- prologue: weights, bias, indices ----
            wt = const_pool.tile([E, K, E], dt.bfloat16)   # W_k^T, [i, k, o]
            nc.sync.dma_start(wt[:], wt_d.ap())

            bias = const_pool.tile([E, 1], dt.float32)
            nc.sync.dma_start(bias[:], convb_d.ap())

            idx_sb = const_pool.tile([128, NCHUNK * IDXCOL], dt.int16)
            nc.sync.dma_start(idx_sb[:], ids_d.ap())

            # ---- main loop ----
            for c in range(NCHUNK):
                et = et_pool.tile([128, 1, IDX_CHUNK], dt.bfloat16)
                for s in range(NSUB):
                    g = c * NSUB + s
                    nc.gpsimd.dma_gather(
                        et[:, :, s * G_IDX:(s + 1) * G_IDX], emb_d.ap(),
                        idx_sb[:, g * G_IDX // 16:(g + 1) * G_IDX // 16],
                        G_IDX, G_IDX, E,
                        transpose=True, queue_num=g % NQUEUES)
                win = et[:, 0, :].rearrange("p (n s) -> p n s", s=SLOTS)

                y_c = y_pool.tile([128, CHUNK_N], dt.float32)
                for g in range(NGROUP):
                    ps = ps_pool.tile([128, GROUP_N * C], dt.float32)
                    for k in range(K):
                        rhs = win[:, g * GROUP_N:(g + 1) * GROUP_N, k:k + C]
                        nc.tensor.matmul(ps[:], lhsT=wt[:, k, :], rhs=rhs,
                                         start=(k == 0), stop=(k == K - 1))
                    nc.vector.tensor_reduce(
                        out=y_c[:, g * GROUP_N:(g + 1) * GROUP_N],
                        in_=ps[:].rearrange("p (n c) -> p n c", c=C),
                        axis=mybir.AxisListType.X,
                        op=mybir.AluOpType.max,
                    )
                nc.scalar.add(y_c[:], y_c[:], bias[:, 0:1])
                nc.sync.dma_start(out_d.ap()[:, c * CHUNK_N:(c + 1) * CHUNK_N],
                                  y_c[:])

    nc.compile()
    return nc


def _wrap_idxs(flat: np.ndarray) -> np.ndarray:
    """SWDGE gather index layout: index i at [i % 16, i // 16], the
    16-partition block replicated across all 8 16-partition groups."""
    block = flat.reshape(-1, 16).T.astype(np.int16)     # [16, n/16]
    return np.tile(block, (8, 1))                       # [128, n/16]


def prep_core_inputs(ids_core: np.ndarray, emb_bf16: np.ndarray,
                     wt_bf16: np.ndarray, conv_b: np.ndarray) -> dict:
    """ids_core: [B_SH, W, C] int for this core."""
    ids = ids_core.reshape(N, C).astype(np.int16)
    padded = np.full((N, SLOTS), VZERO, dtype=np.int16)
    padded[:, PAD:PAD + C] = ids
    return {
        "ids": _wrap_idxs(padded.reshape(-1)),
        "embtab": emb_bf16,
        "wt": wt_bf16,
        "convb": np.ascontiguousarray(conv_b.reshape(E, 1),
                                      dtype=np.float32),
    }


_prog_cache = {}


def kernel(input, lengths, emb_weight, conv_w, conv_b, _trace=False):
    input = np.asarray(input)
    emb_weight = np.asarray(emb_weight, dtype=np.float32)
    conv_w = np.asarray(conv_w, dtype=np.float32)
    conv_b = np.asarray(conv_b, dtype=np.float32)

    if "nc" not in _prog_cache:
        _prog_cache["nc"] = build_program()
    nc = _prog_cache["nc"]

    bf16 = dt.np(dt.bfloat16)
    emb_bf16 = np.zeros((VROWS, E), dtype=bf16)
    emb_bf16[:V] = emb_weight.astype(bf16)
    # lhsT for tap k: wt[i, k*E + o] = conv_w[o, i, k]
    wt_bf16 = np.ascontiguousarray(
        conv_w.transpose(1, 2, 0).reshape(E, K * E)).astype(bf16)

    core_ids = list(range(NCORES))
    in_maps = [
        prep_core_inputs(input[i * B_SH:(i + 1) * B_SH], emb_bf16, wt_bf16,
                         conv_b)
        for i in core_ids
    ]
    res = run_bass_kernel_spmd(nc, in_maps, core_ids, trace=_trace)
    out = np.concatenate(
        [res.results[i]["out"].T.reshape(B_SH, W, E) for i in core_ids],
        axis=0).astype(np.float32)
    if _trace:
        kernel.last_exec_time_ns = res.exec_time_ns
        kernel.last_results = res
    return out

